# revision 12
# baseline (speedup 1.0000x reference)
"""CoordinatesToSpikes on 8 TRN2 NeuronCores.

Reference semantics: times = T_EARLY + cv * (T_LATE - T_EARLY);
idx = round(times / DT); spikes = one-hot along a dense time axis of
length 1000 (each (b, c) pair scatters exactly one 1.0, so the scatter
is a pure one-hot materialization: out[b, t, c] = (idx[b, c] == t)).

Strategy (data-parallel over batch, 256 -> 8 x 32):
  - Host computes idx bit-exactly in fp32 (tiny: 64K elements); values
    are exact integers in [2, 800].
  - On device, SBUF partition p covers batch b = p//4, time-quarter
    tg = p%4 (250 time rows each) so every partition's slice of the
    output is one contiguous 250KB DRAM range -> 10KB DMA descriptors
    (1KB descriptors cap a single HWDGE ring at ~115 GB/s; 10KB ones
    are SDMA-engine-bound at full rate).
  - One K=34 PE matmul builds diff[p, f] = idx[b, f%256] - tg*250
    - f//256 for all partitions (selector rows + folded time base).
    - Then each of 25 chunks (10 time rows) is one DVE compare
    diff == 10*d producing the one-hot tile [128, 2560], DMA-stored as
    a 1.25MB transfer with 10KB contiguous per partition, alternating
    between the two HWDGE rings (sync + scalar engines).
  - Output is write-only, 32.8 MB per core => memory(store)-roofline
    (~92us at 358 GB/s per-core HBM).
"""

import numpy as np
from contextlib import ExitStack

import concourse.bass as bass
import concourse.tile as tile
from concourse import bacc, mybir
from concourse.bass_utils import run_bass_kernel_spmd

F32 = mybir.dt.float32

B, C, SEQ = 256, 256, 1000
NCORES = 8
BSH = B // NCORES          # 32 batches per core
TG = 4                     # time quarters per batch (partition = b*4+tg)
TQ = SEQ // TG             # 250 time rows per quarter
TROWS = 10                 # time rows per chunk
ND = TQ // TROWS           # 25 chunks
FREE = TROWS * C           # 2560 free elements per tile (10KB)
K = BSH + 2                # matmul contraction: 32 selector rows + 2 aux

T_EARLY = np.float32(2e-06)
T_LATE_MINUS_EARLY = np.float32(0.0008 - 2e-06)
DT = np.float32(1e-06)

_compiled = None


def _build():
    nc = bacc.Bacc("TRN2", target_bir_lowering=False, debug=False,
                   num_devices=NCORES)
    F16 = mybir.dt.float16
    idx_d = nc.dram_tensor("idx", [BSH, C], F16, kind="ExternalInput")
    mat_d = nc.dram_tensor("mat", [K, 128], F16, kind="ExternalInput")
    aux_d = nc.dram_tensor("aux", [2, FREE], F16, kind="ExternalInput")
    out_d = nc.dram_tensor("out", [BSH, SEQ, C], F32, kind="ExternalOutput")
    # [128 partitions (b,tg) @ 250KB stride, 25 chunks, 2560 contiguous]
    out_v = out_d.ap().rearrange(
        "b (tg d t) c -> (b tg) d (t c)", tg=TG, d=ND, t=TROWS)

    with ExitStack() as ctx:
        tc = ctx.enter_context(tile.TileContext(nc))
        const = ctx.enter_context(tc.tile_pool(name="const", bufs=1))
        dpool = ctx.enter_context(tc.tile_pool(name="diff", bufs=1))
        pspool = ctx.enter_context(
            tc.tile_pool(name="ps", bufs=1, space="PSUM"))
        outp = ctx.enter_context(tc.tile_pool(name="outp", bufs=4))

        # rhs rows 0..31: idx rows tiled 10x along free; rows 32,33: aux
        # (t_local pattern, ones). fp16: all values are integers <= 2048,
        # exactly representable, and PE accumulates into fp32 -> exact.
        rhs = const.tile([K, FREE], F16)
        nc.sync.dma_start(
            rhs[0:BSH, :].rearrange("k (r c) -> k r c", r=TROWS),
            idx_d.ap().unsqueeze(1).broadcast_to((BSH, TROWS, C)))
        nc.sync.dma_start(rhs[BSH:K, :], aux_d.ap())
        mat = const.tile([K, 128], F16)
        nc.scalar.dma_start(mat[:], mat_d.ap())

        # diff[p, f] = idx[p//4, f%256] - (p%4)*250 - f//256
        ps = pspool.tile([128, FREE], F32)
        for j in range(FREE // 512):
            nc.tensor.matmul(ps[:, j * 512:(j + 1) * 512], mat[:],
                             rhs[:, j * 512:(j + 1) * 512],
                             start=True, stop=True)
        diff = dpool.tile([128, FREE], F32)
        nc.scalar.copy(diff[:], ps[:])

        # First two chunks read PSUM directly (skips waiting on the
        # PSUM->SBUF copy); the rest read the SBUF copy at 2x DVE rate.
        for d in range(ND):
            src = ps if d < 2 else diff
            ot = outp.tile([128, FREE], F32)
            nc.vector.tensor_scalar(
                ot[:], src[:], float(TROWS * d), None,
                mybir.AluOpType.is_equal)
            eng = nc.sync if d % 2 == 0 else nc.scalar
            eng.dma_start(out_v[:, d, :], ot[:])
    nc.compile()
    return nc


def _host_idx(coordinate_values: np.ndarray) -> np.ndarray:
    """Bit-exact fp32 mirror of the reference index computation."""
    cv = np.ascontiguousarray(coordinate_values, dtype=np.float32)
    times = T_EARLY + cv * T_LATE_MINUS_EARLY
    return np.rint(times / DT).astype(np.float32)


def _host_consts():
    p = np.arange(128)
    mat = np.zeros((K, 128), np.float16)
    mat[p // TG, p] = 1.0                      # selector rows
    mat[BSH, :] = -1.0                         # coefficient for t_local
    mat[BSH + 1, :] = -(p % TG).astype(np.float16) * TQ  # -tg*250
    aux = np.empty((2, FREE), np.float16)
    aux[0] = np.repeat(np.arange(TROWS, dtype=np.float16), C)  # f//256
    aux[1] = 1.0
    return mat, aux


def _in_maps(coordinate_values: np.ndarray) -> list[dict]:
    idxf = _host_idx(coordinate_values)                      # (256, 256)
    mat, aux = _host_consts()
    return [
        {"idx": np.ascontiguousarray(
            idxf[m * BSH:(m + 1) * BSH]).astype(np.float16),
         "mat": mat, "aux": aux}
        for m in range(NCORES)
    ]


def kernel(coordinate_values: np.ndarray) -> np.ndarray:
    global _compiled
    if _compiled is None:
        _compiled = _build()
    res = run_bass_kernel_spmd(
        _compiled, _in_maps(coordinate_values),
        core_ids=list(range(NCORES)))
    return np.concatenate([r["out"] for r in res.results], axis=0)
